# revision 5
# baseline (speedup 1.0000x reference)
"""Trainium2 Bass kernel for nn_AutoEncoder (VAE-style autoencoder, pure data parallel).

Sharding: batch dim B=131072 split across 8 NeuronCores (16384 each); tiny MLP
weights replicated.  Per core, everything is laid out "b-coarse" in SBUF
(partition p holds 128 consecutive batch rows' feature vectors contiguously) so
all HBM traffic is large contiguous descriptors; PE-transposes convert between
that layout and the feature-major layout matmuls need.  Batch samples are
packed 6-per-matmul with block-diagonal weights so PE columns carry 6 samples.

Self-contained: hardcodes all shapes; only needs concourse (bass) + numpy/jax.
"""

import os
import sys
import numpy as np

for _p in ("/opt/trn_rl_repo", "/root/.axon_site/_ro/trn_rl_repo"):
    if os.path.isdir(_p) and _p not in sys.path:
        sys.path.insert(0, _p)

import ml_dtypes


def _install_ntff_hook():
    """The agent image's antenv lacks axon_hooks; shim it so trace=True works."""
    try:
        import antenv.axon_hooks  # noqa: F401
        return
    except ImportError:
        pass
    import types
    import antenv
    mod = types.ModuleType("antenv.axon_hooks")
    store = [None]
    mod.set_axon_ntff_profile_hook = lambda h: store.__setitem__(0, h)
    mod.get_axon_ntff_profile_hook = lambda: store[0]
    sys.modules["antenv.axon_hooks"] = mod
    antenv.axon_hooks = mod
    try:
        from trn_agent_boot.trn_boot import _ntff_profile_via_ctypes
        so = "/opt/axon/libaxon_pjrt.so"
        if os.path.exists(so):
            store[0] = _ntff_profile_via_ctypes(so)
    except Exception:
        pass


_install_ntff_hook()

import concourse.bass as bass
import concourse.mybir as mybir
from concourse import bacc
from concourse.tile import TileContext
from concourse.bass_utils import run_bass_kernel_spmd

F32 = mybir.dt.float32
BF16 = mybir.dt.bfloat16

# ---------------- problem constants ----------------
B_FULL = 131072
NCORES = 8
BS = B_FULL // NCORES        # 16384 batch rows per core
P = 128                      # SBUF partitions
BI = BS // P                 # 128 batch rows per partition (b-coarse layout)
E = 10                       # encoded size
D = 20                       # data size
MC = 50                      # monte-carlo samples
BN_EPS = 1e-5
LOGVAR_OFFSET = 0.05
ENC_DIMS = [20, 20, 16, 14, 12, 10]
DEC_DIMS = [10, 12, 14, 16, 20, 20]

NB = 6                       # batch samples per block-diag matmul (main chunks)
# chunk c covers b_in (within-partition batch index) range [b0, b0+nb)
CHUNKS = [(c, 6, 6 * c) for c in range(21)] + [(21, 2, 126)]


def colgroups():
    gs = []
    for g in range(5):
        gs.append(dict(chunks=[4 * g, 4 * g + 1, 4 * g + 2, 4 * g + 3],
                       nb=6, N=512, col0=512 * g))
    gs.append(dict(chunks=[20], nb=6, N=128, col0=2560))
    gs.append(dict(chunks=[21], nb=2, N=128, col0=2688))
    return gs


GROUPS = colgroups()
FM_COLS = 2816               # 5*512 + 128 + 128 (total fm columns incl. tail group)


# ---------------- host-side weight prep ----------------
class Pack:
    """Packs many small [rows<=128, cols] matrices into one [128, C] array."""

    def __init__(self, np_dtype):
        self.np_dtype = np_dtype
        self.cols = 0
        self.items = {}

    def add(self, name, arr):
        arr = np.asarray(arr)
        r, c = arr.shape
        assert r <= 128
        self.items[name] = (self.cols, r, c, arr)
        self.cols += c

    def finalize(self):
        out = np.zeros((128, self.cols), dtype=self.np_dtype)
        for c0, r, c, a in self.items.values():
            out[:r, c0:c0 + c] = a.astype(self.np_dtype)
        return out

    def ap(self, tile, name):
        c0, r, c, _ = self.items[name]
        return tile[0:r, c0:c0 + c]


def fold_bn(params):
    """Linear+BN(eval) -> single linear.  Returns [(W[fin,fout], b[fout])...]"""
    out = []
    n = len(params)
    for i, p in enumerate(params):
        W = np.asarray(p["w"], np.float64)
        b = np.asarray(p["b"], np.float64)
        if i < n - 1:
            sc = np.asarray(p["gamma"], np.float64) / np.sqrt(
                np.asarray(p["rv"], np.float64) + BN_EPS)
            b = b * sc + (np.asarray(p["beta"], np.float64)
                          - np.asarray(p["rm"], np.float64) * sc)
            W = W * sc[None, :]
        out.append((W.astype(np.float32), b.astype(np.float32)))
    return out


def bd(W, nb):
    return np.kron(np.eye(nb, dtype=W.dtype), W)


def make_packs(enc_mean, enc_var, dec_mean, np_dt_dec):
    em_l, ev_l, dm_l = fold_bn(enc_mean), fold_bn(enc_var), fold_bn(dec_mean)
    p32 = Pack(np.float32)
    pD = Pack(np_dt_dec)
    for nb in (6, 2):
        for tag, layers in (("em", em_l), ("ev", ev_l)):
            for li, (W, b) in enumerate(layers):
                p32.add(f"{tag}{li}n{nb}", bd(W, nb))
                p32.add(f"{tag}{li}n{nb}_b", np.tile(b, nb)[:, None])
        for li, (W, b) in enumerate(dm_l):
            pD.add(f"dec{li}n{nb}", bd(W, nb))
            p32.add(f"dec{li}n{nb}_b", np.tile(b, nb)[:, None])
    p32.add("id", np.eye(128, dtype=np.float32))
    pD.add("id", np.eye(128, dtype=np.float32))
    return p32, pD


# ---------------- kernel builder ----------------
def build_kernel(p32, pD, dt_dec=F32, mc_n=MC):
    """dt_dec: dtype for z path + decoder matmuls/activations (F32 or BF16)."""
    DT = dt_dec
    cast = DT != F32
    nc = bacc.Bacc()

    x_in = nc.declare_dram_parameter("x_s", [BS, D], F32, isOutput=False)
    eps_in = nc.declare_dram_parameter("eps_s", [mc_n, BS, E], F32, isOutput=False)
    w32_in = nc.declare_dram_parameter("w32", [128, p32.cols], F32, isOutput=False)
    wd_in = nc.declare_dram_parameter("wd", [128, pD.cols], DT, isOutput=False)
    em_out = nc.declare_dram_parameter("em_o", [BS, E], F32, isOutput=True)
    ev_out = nc.declare_dram_parameter("ev_o", [BS, E], F32, isOutput=True)
    z_out = nc.declare_dram_parameter("z_o", [mc_n, BS, E], F32, isOutput=True)
    xp_out = nc.declare_dram_parameter("xp_o", [mc_n, BS, D], F32, isOutput=True)

    AF = mybir.ActivationFunctionType
    OP = mybir.AluOpType

    with TileContext(nc) as tc:
        with (
            tc.tile_pool(name="wp", bufs=1) as wp,
            tc.tile_pool(name="sb", bufs=2) as sb,
            tc.tile_pool(name="pt", bufs=3, space="PSUM") as pt,   # transposes
            tc.tile_pool(name="pm", bufs=3, space="PSUM") as pm,   # matmuls
        ):
            w32 = wp.tile([128, p32.cols], F32)
            nc.sync.dma_start(w32, w32_in[:, :])
            wd = wp.tile([128, pD.cols], DT)
            nc.sync.dma_start(wd, wd_in[:, :])

            def W32(name):
                return p32.ap(w32, name)

            def WD(name):
                return pD.ap(wd, name)

            id32 = W32("id")
            idD = pD.ap(wd, "id")

            # round-robin for ops that can go on either DVE or ACT
            rr = [0]

            def evac_copy(out, in_):
                if rr[0] & 1:
                    nc.scalar.activation(out, in_, AF.Copy)
                else:
                    nc.vector.tensor_copy(out, in_)
                rr[0] += 1

            def evac_relu(out, psum, bias_ap, on_act):
                if on_act:
                    nc.scalar.activation(out, psum, AF.Relu, bias=bias_ap)
                else:
                    nc.vector.tensor_scalar(out, psum, bias_ap, 0.0, OP.add, OP.max)

            # ---------------- encoder (fp32) ----------------
            xld = sb.tile([P, BI * D], F32, bufs=1)
            nc.sync.dma_start(xld, x_in.rearrange("(p bi) e -> p (bi e)", p=P))

            em_fm = wp.tile([60, FM_COLS], F32)
            ev_fm = wp.tile([60, FM_COLS], F32)
            s_fm = wp.tile([60, FM_COLS], F32)

            for grp in GROUPS:
                nb, N = grp["nb"], grp["N"]
                col0 = grp["col0"]
                # x transpose: b-coarse -> feature-major
                xT = pt.tile([120, 512], F32, tag="tp")
                for i, c in enumerate(grp["chunks"]):
                    nc.tensor.transpose(
                        xT[0:nb * D, 128 * i:128 * i + 128],
                        xld[:, D * NB * c: D * NB * c + D * nb],
                        id32)
                xfm = sb.tile([120, 512], F32, tag="xfm")
                evac_copy(xfm[0:nb * D, 0:N], xT[0:nb * D, 0:N])

                h = {"em": xfm[0:nb * D, 0:N], "ev": xfm[0:nb * D, 0:N]}
                for li in range(5):
                    fin, fout = ENC_DIMS[li], ENC_DIMS[li + 1]
                    for ch in ("em", "ev"):
                        ps_t = pm.tile([120, 512], F32, tag="mm")
                        mm = ps_t[0:nb * fout, 0:N]
                        nc.tensor.matmul(mm, W32(f"{ch}{li}n{nb}"),
                                         h[ch][0:nb * fin, 0:N],
                                         start=True, stop=True)
                        bias = W32(f"{ch}{li}n{nb}_b")
                        if li < 4:
                            hn = sb.tile([120, 512], F32, tag=f"h{ch}")
                            evac_relu(hn[0:nb * fout, 0:N], mm, bias,
                                      on_act=(li + (ch == "ev")) & 1)
                            h[ch] = hn[0:nb * fout, 0:N]
                        elif ch == "em":
                            nc.vector.tensor_scalar(
                                em_fm[0:nb * E, col0:col0 + N], mm, bias,
                                None, OP.add)
                        else:
                            sg = sb.tile([60, 512], F32, tag="sg")
                            nc.scalar.activation(sg[0:nb * E, 0:N], mm,
                                                 AF.Sigmoid, bias=bias)
                            nc.vector.tensor_scalar(
                                ev_fm[0:nb * E, col0:col0 + N],
                                sg[0:nb * E, 0:N], LOGVAR_OFFSET, None, OP.add)

            # sqrt pass (separate so ACT table set switches only twice)
            for grp in GROUPS:
                nb, N, col0 = grp["nb"], grp["N"], grp["col0"]
                nc.scalar.activation(s_fm[0:nb * E, col0:col0 + N],
                                     ev_fm[0:nb * E, col0:col0 + N], AF.Sqrt)

            # transpose back to b-coarse layout + store em/ev; keep em/s
            em_bm = wp.tile([P, BI * E], F32)
            ev_bm = sb.tile([P, BI * E], F32, bufs=1)
            s_bm = wp.tile([P, BI * E], DT)
            for grp in GROUPS:
                nb, N, col0 = grp["nb"], grp["N"], grp["col0"]
                for src, dst in ((em_fm, em_bm), (ev_fm, ev_bm), (s_fm, s_bm)):
                    tb = pt.tile([128, 512], F32, tag="tp")
                    for i, c in enumerate(grp["chunks"]):
                        nc.tensor.transpose(
                            tb[:, 60 * i:60 * i + E * nb],
                            src[0:E * nb, col0 + 128 * i:col0 + 128 * (i + 1)],
                            id32[0:E * nb, 0:E * nb])
                    ci = E * NB * grp["chunks"][0]
                    w = E * nb * len(grp["chunks"])
                    evac_copy(dst[:, ci:ci + w], tb[:, 0:w])
            nc.sync.dma_start(em_out.rearrange("(p bi) e -> p (bi e)", p=P), em_bm)
            nc.sync.dma_start(ev_out.rearrange("(p bi) e -> p (bi e)", p=P), ev_bm)

            em_bmz = em_bm
            if cast:
                em_bmz = wp.tile([P, BI * E], DT)
                nc.vector.tensor_copy(em_bmz, em_bm)

            dma_c = nc.gpsimd if cast else nc.sync

            # ---------------- decoder (per MC sample) ----------------
            for mc in range(mc_n):
                epz = sb.tile([P, BI * E], DT, tag="epz")
                dma_c.dma_start(epz, eps_in[mc].rearrange("(p bi) e -> p (bi e)", p=P))
                sepz = sb.tile([P, BI * E], DT, tag="sepz")
                nc.vector.tensor_tensor(sepz, s_bm, epz, OP.mult)
                zt = sb.tile([P, BI * E], DT, tag="zt")
                nc.vector.tensor_tensor(zt, em_bmz, sepz, OP.add)
                dma_c.dma_start(z_out[mc].rearrange("(p bi) e -> p (bi e)", p=P), zt)

                xp_bm = sb.tile([P, BI * D], DT, tag="xp_bm")
                for grp in GROUPS:
                    nb, N, col0 = grp["nb"], grp["N"], grp["col0"]
                    chunks = grp["chunks"]
                    # z: b-coarse -> feature-major
                    zT = pt.tile([60, 512], DT, tag="tp")
                    for i, c in enumerate(chunks):
                        nc.tensor.transpose(
                            zT[0:E * nb, 128 * i:128 * i + 128],
                            zt[:, E * NB * c: E * NB * c + E * nb], idD)
                    zfm = sb.tile([60, 512], DT, tag="zfm")
                    evac_copy(zfm[0:E * nb, 0:N], zT[0:E * nb, 0:N])

                    cur = zfm[0:E * nb, 0:N]
                    for li in range(5):
                        fin, fout = DEC_DIMS[li], DEC_DIMS[li + 1]
                        ps_t = pm.tile([120, 512], F32, tag="mm")
                        mm = ps_t[0:nb * fout, 0:N]
                        nc.tensor.matmul(mm, WD(f"dec{li}n{nb}"),
                                         cur[0:nb * fin, 0:N],
                                         start=True, stop=True)
                        bias = W32(f"dec{li}n{nb}_b")
                        if li < 4:
                            hn = sb.tile([120, 512], DT, tag=f"dh{li}")
                            evac_relu(hn[0:nb * fout, 0:N], mm, bias,
                                      on_act=li & 1)
                            cur = hn[0:nb * fout, 0:N]
                        else:
                            xpfm = sb.tile([120, 512], DT, tag="xpfm")
                            nc.scalar.activation(xpfm[0:nb * D, 0:N], mm,
                                                 AF.Sigmoid, bias=bias)
                    # x_pred: feature-major -> b-coarse
                    xT2 = pt.tile([128, 512], DT, tag="tp")
                    for i, c in enumerate(chunks):
                        nc.tensor.transpose(
                            xT2[:, 120 * i:120 * i + D * nb],
                            xpfm[0:D * nb, 128 * i:128 * (i + 1)],
                            idD[0:D * nb, 0:D * nb])
                    ci = D * NB * chunks[0]
                    w = D * nb * len(chunks)
                    evac_copy(xp_bm[:, ci:ci + w], xT2[:, 0:w])
                dma_c.dma_start(xp_out[mc].rearrange("(p bi) e -> p (bi e)", p=P),
                                xp_bm)
    nc.finalize()
    return nc


# ---------------- host entry ----------------
def _run(x, enc_mean, enc_var, dec_mean, eps, dt="bf16", mc_n=MC,
         trace=False, trace_kwargs=None):
    np_dt = {"f32": np.float32, "bf16": ml_dtypes.bfloat16}[dt]
    dt_dec = {"f32": F32, "bf16": BF16}[dt]
    p32, pD = make_packs(enc_mean, enc_var, dec_mean, np_dt)
    nc = build_kernel(p32, pD, dt_dec=dt_dec, mc_n=mc_n)

    x = np.asarray(x, np.float32)
    eps = np.asarray(eps, np.float32)
    w32_a = p32.finalize()
    wd_a = pD.finalize()
    in_maps = []
    for c in range(NCORES):
        sl = slice(c * BS, (c + 1) * BS)
        in_maps.append({
            "x_s": np.ascontiguousarray(x[sl]),
            "eps_s": np.ascontiguousarray(eps[:mc_n, sl]),
            "w32": w32_a,
            "wd": wd_a,
        })
    res = run_bass_kernel_spmd(nc, in_maps, list(range(NCORES)),
                               trace=trace, **(trace_kwargs or {}))
    em = np.concatenate([r["em_o"] for r in res.results], axis=0)
    ev = np.concatenate([r["ev_o"] for r in res.results], axis=0)
    z = np.concatenate([r["z_o"] for r in res.results], axis=1)
    xp = np.concatenate([r["xp_o"] for r in res.results], axis=1)
    return (em, ev, z, xp), res


def kernel(x, enc_mean, enc_var, dec_mean, eps):
    outs, _ = _run(x, enc_mean, enc_var, dec_mean, eps,
                   dt=os.environ.get("KERNEL_DT", "bf16"))
    return outs


# revision 8
# speedup vs baseline: 2.2414x; 2.2414x over previous
"""Trainium2 Bass kernel for nn_AutoEncoder (VAE-style autoencoder, pure data parallel).

Sharding: batch dim B=131072 split across 8 NeuronCores (16384 each); tiny MLP
weights replicated.  Per core, everything is laid out "b-coarse" in SBUF
(partition p holds 128 consecutive batch rows' feature vectors contiguously) so
all HBM traffic is large contiguous descriptors; PE-transposes convert between
that layout and the feature-major layout matmuls need.  Batch samples are
packed 6..12-per-matmul with block-diagonal weights so PE columns carry many
samples.  Decoder runs layer-major so the PE sees long same-weight matmul runs
(stays HAM-warm); elementwise work is split across ScalarE and VectorE.

Self-contained: hardcodes all shapes; only needs concourse (bass) + numpy.
"""

import os
import sys
import numpy as np

for _p in ("/opt/trn_rl_repo", "/root/.axon_site/_ro/trn_rl_repo"):
    if os.path.isdir(_p) and _p not in sys.path:
        sys.path.insert(0, _p)

import ml_dtypes


def _install_ntff_hook():
    """The agent image's antenv lacks axon_hooks; shim it so trace=True works."""
    try:
        import antenv.axon_hooks  # noqa: F401
        return
    except ImportError:
        pass
    import types
    import antenv
    mod = types.ModuleType("antenv.axon_hooks")
    store = [None]
    mod.set_axon_ntff_profile_hook = lambda h: store.__setitem__(0, h)
    mod.get_axon_ntff_profile_hook = lambda: store[0]
    sys.modules["antenv.axon_hooks"] = mod
    antenv.axon_hooks = mod
    try:
        from trn_agent_boot.trn_boot import _ntff_profile_via_ctypes
        so = "/opt/axon/libaxon_pjrt.so"
        if os.path.exists(so):
            store[0] = _ntff_profile_via_ctypes(so)
    except Exception:
        pass


_install_ntff_hook()

import concourse.bass as bass  # noqa: E402
import concourse.mybir as mybir  # noqa: E402
from concourse import bacc  # noqa: E402
from concourse.tile import TileContext  # noqa: E402
from concourse.bass_utils import run_bass_kernel_spmd  # noqa: E402

F32 = mybir.dt.float32
BF16 = mybir.dt.bfloat16

# ---------------- problem constants ----------------
B_FULL = 131072
NCORES = 8
BS = B_FULL // NCORES        # 16384 batch rows per core
P = 128                      # SBUF partitions
BI = BS // P                 # 128 batch rows per partition (b-coarse layout)
E = 10                       # encoded size
D = 20                       # data size
MC = 50                      # monte-carlo samples
BN_EPS = 1e-5
LOGVAR_OFFSET = 0.05
ENC_DIMS = [20, 20, 16, 14, 12, 10]
DEC_DIMS = [10, 12, 14, 16, 20, 20]

NB = 6                       # samples per block in block-diag weights
# chunk c covers b_in range [6c, 6c+6) for c in 0..20; chunk 21 = tail [126,128)
# encoder column groups (feature-major tiles of 4 transposed 128-col chunks)
ENC_GROUPS = (
    [dict(chunks=list(range(4 * g, 4 * g + 4)), nb=6, N=512, col0=512 * g)
     for g in range(5)]
    + [dict(chunks=[20], nb=6, N=128, col0=2560),
       dict(chunks=[21], nb=2, N=128, col0=2688)]
)
FM_COLS = 2816

# decoder: wide z-transposes j=0..9 cover b_in [12j,12j+12) (= chunks 2j,2j+1)
# L1 is done on 12-sample columns via an M-split block-diag-12 weight; the two
# halves of its output are 6-block tiles whose chunk lists stride by 2.
DEC_WGROUPS = [(0, 4), (4, 4), (8, 2)]   # (first wide-transpose j, count)


def dec_tiles():
    ts = []
    for wg, (j0, nj) in enumerate(DEC_WGROUPS):
        for half in (0, 1):
            ts.append(dict(chunks=[2 * (j0 + i) + half for i in range(nj)],
                           nb=6, N=128 * nj, src=("wide", wg, half)))
    ts.append(dict(chunks=[20], nb=6, N=128, src=("n20",)))
    ts.append(dict(chunks=[21], nb=2, N=128, src=("tail",)))
    return ts


DEC_TILES = dec_tiles()


# ---------------- host-side weight prep ----------------
class Pack:
    """Packs many small [rows<=128, cols] matrices into one [128, C] array."""

    def __init__(self, np_dtype):
        self.np_dtype = np_dtype
        self.cols = 0
        self.items = {}

    def add(self, name, arr):
        arr = np.asarray(arr)
        r, c = arr.shape
        assert r <= 128
        self.items[name] = (self.cols, r, c, arr)
        self.cols += c

    def finalize(self):
        out = np.zeros((128, self.cols), dtype=self.np_dtype)
        for c0, r, c, a in self.items.values():
            out[:r, c0:c0 + c] = a.astype(self.np_dtype)
        return out

    def ap(self, tile, name):
        c0, r, c, _ = self.items[name]
        return tile[0:r, c0:c0 + c]


def fold_bn(params):
    """Linear+BN(eval) -> single linear.  Returns [(W[fin,fout], b[fout])...]"""
    out = []
    n = len(params)
    for i, p in enumerate(params):
        W = np.asarray(p["w"], np.float64)
        b = np.asarray(p["b"], np.float64)
        if i < n - 1:
            sc = np.asarray(p["gamma"], np.float64) / np.sqrt(
                np.asarray(p["rv"], np.float64) + BN_EPS)
            b = b * sc + (np.asarray(p["beta"], np.float64)
                          - np.asarray(p["rm"], np.float64) * sc)
            W = W * sc[None, :]
        out.append((W.astype(np.float32), b.astype(np.float32)))
    return out


def bd(W, nb):
    return np.kron(np.eye(nb, dtype=W.dtype), W)


def make_packs(enc_mean, enc_var, dec_mean, np_dt_dec):
    em_l, ev_l, dm_l = fold_bn(enc_mean), fold_bn(enc_var), fold_bn(dec_mean)
    p32 = Pack(np.float32)    # biases + fp32 identity
    pD = Pack(np_dt_dec)      # all matmul weights + DT identity
    for nb in (6, 2):
        for tag, layers in (("em", em_l), ("ev", ev_l)):
            for li, (W, b) in enumerate(layers):
                pD.add(f"{tag}{li}n{nb}", bd(W, nb))
                p32.add(f"{tag}{li}n{nb}_b", np.tile(b, nb)[:, None])
        for li, (W, b) in enumerate(dm_l):
            pD.add(f"dec{li}n{nb}", bd(W, nb))
            p32.add(f"dec{li}n{nb}_b", np.tile(b, nb)[:, None])
    pD.add("dec0n12", bd(dm_l[0][0], 12))     # [120, 144] for the L1 M-split
    p32.add("id", np.eye(128, dtype=np.float32))
    pD.add("id", np.eye(128, dtype=np.float32))
    return p32, pD


# ---------------- kernel builder ----------------
def build_kernel(p32, pD, dt_dec=BF16, mc_n=MC):
    DT = dt_dec
    cast = DT != F32
    nc = bacc.Bacc()

    x_in = nc.declare_dram_parameter("x_s", [BS, D], F32, isOutput=False)
    eps_in = nc.declare_dram_parameter("eps_s", [mc_n, BS, E], F32, isOutput=False)
    w32_in = nc.declare_dram_parameter("w32", [128, p32.cols], F32, isOutput=False)
    wd_in = nc.declare_dram_parameter("wd", [128, pD.cols], DT, isOutput=False)
    em_out = nc.declare_dram_parameter("em_o", [BS, E], F32, isOutput=True)
    ev_out = nc.declare_dram_parameter("ev_o", [BS, E], F32, isOutput=True)
    z_out = nc.declare_dram_parameter("z_o", [mc_n, BS, E], F32, isOutput=True)
    xp_out = nc.declare_dram_parameter("xp_o", [mc_n, BS, D], F32, isOutput=True)

    AF = mybir.ActivationFunctionType
    OP = mybir.AluOpType

    with TileContext(nc) as tc:
        with (
            tc.tile_pool(name="wp", bufs=1) as wp,
            tc.tile_pool(name="sb", bufs=2) as sb,
            tc.tile_pool(name="pt", bufs=3, space="PSUM") as pt,   # transposes
            tc.tile_pool(name="pm", bufs=4, space="PSUM") as pm,   # matmuls
        ):
            w32 = wp.tile([128, p32.cols], F32)
            nc.sync.dma_start(w32, w32_in[:, :])
            wd = wp.tile([128, pD.cols], DT)
            nc.sync.dma_start(wd, wd_in[:, :])

            def W(name):
                return pD.ap(wd, name)

            def B(name):
                return p32.ap(w32, name)

            idD = pD.ap(wd, "id")

            rr = [0]

            def evac_copy(out, in_):
                # pure PSUM->SBUF copies; keep u32/bf16 bit copies off ACT
                if rr[0] % 3 == 2:
                    nc.scalar.activation(out, in_, AF.Copy)
                else:
                    nc.vector.tensor_copy(out, in_)
                rr[0] += 1

            def evac_relu(out, psum, bias_ap, on_act):
                if on_act:
                    nc.scalar.activation(out, psum, AF.Relu, bias=bias_ap)
                else:
                    nc.vector.tensor_scalar(out, psum, bias_ap, 0.0, OP.add, OP.max)

            # ---------------- encoder (bf16 matmuls, fp32 outputs) ----------
            xld = sb.tile([P, BI * D], DT, bufs=1)
            nc.gpsimd.dma_start(xld, x_in.rearrange("(p bi) e -> p (bi e)", p=P))

            em_fm = wp.tile([60, FM_COLS], F32)
            ev_fm = wp.tile([60, FM_COLS], F32)
            s_fm = wp.tile([60, FM_COLS], F32)

            for grp in ENC_GROUPS:
                nb, N = grp["nb"], grp["N"]
                col0 = grp["col0"]
                xT = pt.tile([120, 512], DT, tag="tp")
                for i, c in enumerate(grp["chunks"]):
                    nc.tensor.transpose(
                        xT[0:nb * D, 128 * i:128 * i + 128],
                        xld[:, D * NB * c: D * NB * c + D * nb],
                        idD)
                xfm = sb.tile([120, 512], DT, tag="xfm")
                evac_copy(xfm[0:nb * D, 0:N], xT[0:nb * D, 0:N])

                h = {"em": xfm[0:nb * D, 0:N], "ev": xfm[0:nb * D, 0:N]}
                for li in range(5):
                    fin, fout = ENC_DIMS[li], ENC_DIMS[li + 1]
                    for ch in ("em", "ev"):
                        ps_t = pm.tile([120, 512], F32, tag="mm")
                        mm = ps_t[0:nb * fout, 0:N]
                        nc.tensor.matmul(mm, W(f"{ch}{li}n{nb}"),
                                         h[ch][0:nb * fin, 0:N],
                                         start=True, stop=True)
                        bias = B(f"{ch}{li}n{nb}_b")
                        if li < 4:
                            hn = sb.tile([120, 512], DT, tag=f"h{ch}")
                            evac_relu(hn[0:nb * fout, 0:N], mm, bias,
                                      on_act=(li + (ch == "ev")) & 1)
                            h[ch] = hn[0:nb * fout, 0:N]
                        elif ch == "em":
                            nc.vector.tensor_scalar(
                                em_fm[0:nb * E, col0:col0 + N], mm, bias,
                                None, OP.add)
                        else:
                            sg = sb.tile([60, 512], F32, tag="sg")
                            nc.scalar.activation(sg[0:nb * E, 0:N], mm,
                                                 AF.Sigmoid, bias=bias)
                            nc.vector.tensor_scalar(
                                ev_fm[0:nb * E, col0:col0 + N],
                                sg[0:nb * E, 0:N], LOGVAR_OFFSET, None, OP.add)

            # sqrt pass (separate so ACT table set switches only twice)
            for grp in ENC_GROUPS:
                nb, N, col0 = grp["nb"], grp["N"], grp["col0"]
                nc.scalar.activation(s_fm[0:nb * E, col0:col0 + N],
                                     ev_fm[0:nb * E, col0:col0 + N], AF.Sqrt)

            # transpose back to b-coarse layout + store em/ev; keep em/s
            id32 = p32.ap(w32, "id")
            em_bm = wp.tile([P, BI * E], F32)
            ev_bm = sb.tile([P, BI * E], F32, bufs=1)
            s_bm = wp.tile([P, BI * E], DT)
            for grp in ENC_GROUPS:
                nb, N, col0 = grp["nb"], grp["N"], grp["col0"]
                for src, dst in ((em_fm, em_bm), (ev_fm, ev_bm), (s_fm, s_bm)):
                    tb = pt.tile([128, 512], F32, tag="tp")
                    for i, c in enumerate(grp["chunks"]):
                        nc.tensor.transpose(
                            tb[:, 60 * i:60 * i + E * nb],
                            src[0:E * nb, col0 + 128 * i:col0 + 128 * (i + 1)],
                            id32[0:E * nb, 0:E * nb])
                    ci = E * NB * grp["chunks"][0]
                    w_ = E * nb * len(grp["chunks"])
                    evac_copy(dst[:, ci:ci + w_], tb[:, 0:w_])
            nc.sync.dma_start(em_out.rearrange("(p bi) e -> p (bi e)", p=P), em_bm)
            nc.sync.dma_start(ev_out.rearrange("(p bi) e -> p (bi e)", p=P), ev_bm)

            em_bmz = em_bm
            if cast:
                em_bmz = wp.tile([P, BI * E], DT)
                nc.vector.tensor_copy(em_bmz, em_bm)

            dma_c = nc.gpsimd if cast else nc.sync

            # ---------------- decoder (per MC sample) ----------------
            for mc in range(mc_n):
                epz = sb.tile([P, BI * E], DT, tag="epz")
                dma_c.dma_start(epz,
                                eps_in[mc].rearrange("(p bi) e -> p (bi e)", p=P))
                sepz = sb.tile([P, BI * E], DT, tag="sepz")
                nc.vector.tensor_tensor(sepz, s_bm, epz, OP.mult)
                zt = sb.tile([P, BI * E], DT, tag="zt")
                nc.vector.tensor_tensor(zt, em_bmz, sepz, OP.add)
                dma_c.dma_start(z_out[mc].rearrange("(p bi) e -> p (bi e)", p=P),
                                zt)

                # z: b-coarse -> feature-major (10 wide + 1 narrow + 1 tail)
                zfmw = sb.tile([120, 1280], DT, tag="zfmw")
                for wg, (j0, nj) in enumerate(DEC_WGROUPS):
                    zT = pt.tile([120, 512], DT, tag="tp")
                    for i in range(nj):
                        j = j0 + i
                        nc.tensor.transpose(zT[:, 128 * i:128 * (i + 1)],
                                            zt[:, 120 * j:120 * (j + 1)], idD)
                    evac_copy(zfmw[:, 512 * wg:512 * wg + 128 * nj],
                              zT[:, 0:128 * nj])
                zx = pt.tile([80, 512], DT, tag="tp")
                nc.tensor.transpose(zx[0:60, 0:128], zt[:, 1200:1260], idD)
                nc.tensor.transpose(zx[0:20, 128:256], zt[:, 1260:1280], idD)
                zfm20 = sb.tile([60, 256], DT, tag="zfm20")
                evac_copy(zfm20[0:60, 0:128], zx[0:60, 0:128])
                evac_copy(zfm20[0:20, 128:256], zx[0:20, 128:256])

                # L1 (layer-major, M-split block-diag-12 on the wide tiles)
                w12 = W("dec0n12")
                b1 = B("dec0n6_b")
                cur = {}
                for ti, t in enumerate(DEC_TILES):
                    nb, N, src = t["nb"], t["N"], t["src"]
                    ps1 = pm.tile([120, 512], F32, tag="mm")
                    if src[0] == "wide":
                        wg, half = src[1], src[2]
                        nc.tensor.matmul(
                            ps1[0:72, 0:N], w12[:, 72 * half:72 * half + 72],
                            zfmw[0:120, 512 * wg:512 * wg + N],
                            start=True, stop=True)
                        bb = b1
                    elif src[0] == "n20":
                        nc.tensor.matmul(ps1[0:72, 0:N], W("dec0n6"),
                                         zfm20[0:60, 0:128],
                                         start=True, stop=True)
                        bb = b1
                    else:
                        nc.tensor.matmul(ps1[0:24, 0:N], W("dec0n2"),
                                         zfm20[0:20, 128:256],
                                         start=True, stop=True)
                        bb = B("dec0n2_b")
                    hn = sb.tile([120, 512], DT, tag="dh0", bufs=10)
                    evac_relu(hn[0:nb * 12, 0:N], ps1[0:nb * 12, 0:N], bb,
                              on_act=ti & 1)
                    cur[ti] = hn

                for li in range(1, 5):
                    fin, fout = DEC_DIMS[li], DEC_DIMS[li + 1]
                    for ti, t in enumerate(DEC_TILES):
                        nb, N = t["nb"], t["N"]
                        sfx = f"n{nb}"
                        ps_t = pm.tile([120, 512], F32, tag="mm")
                        mm = ps_t[0:nb * fout, 0:N]
                        nc.tensor.matmul(mm, W(f"dec{li}{sfx}"),
                                         cur[ti][0:nb * fin, 0:N],
                                         start=True, stop=True)
                        if li < 4:
                            hn = sb.tile([120, 512], DT, tag=f"dh{li}", bufs=10)
                            evac_relu(hn[0:nb * fout, 0:N], mm,
                                      B(f"dec{li}{sfx}_b"), on_act=(ti + li) & 1)
                            cur[ti] = hn
                        else:
                            xpfm = sb.tile([120, 512], DT, tag="xpfm", bufs=10)
                            nc.scalar.activation(xpfm[0:nb * D, 0:N], mm,
                                                 AF.Sigmoid,
                                                 bias=B(f"dec{li}{sfx}_b"))
                            cur[ti] = xpfm

                # x_pred: feature-major -> b-coarse, store per tile
                xpv = xp_out[mc].rearrange("(p bi) e -> p bi e", p=P)
                for ti, t in enumerate(DEC_TILES):
                    nb, N, chunks = t["nb"], t["N"], t["chunks"]
                    nch = len(chunks)
                    stride = chunks[1] - chunks[0] if nch > 1 else 1
                    xT2 = pt.tile([128, 512], DT, tag="tp")
                    for i in range(nch):
                        nc.tensor.transpose(
                            xT2[:, nb * D * i:nb * D * i + nb * D],
                            cur[ti][0:nb * D, 128 * i:128 * (i + 1)],
                            idD[0:nb * D, 0:nb * D])
                    xpb = sb.tile([128, 512], DT, tag="xpb", bufs=4)
                    evac_copy(xpb[:, 0:nb * D * nch], xT2[:, 0:nb * D * nch])
                    # DRAM dst: per partition, runs of nb consecutive b at
                    # b_in = 6*chunks[0] + 6*stride*i
                    b0 = NB * chunks[0]
                    if nch > 1:
                        dst = xpv[:, b0:b0 + NB * stride * nch, :].rearrange(
                            "p (i r) e -> p i r e", r=NB * stride)[:, :, 0:nb, :]
                        src = xpb[:, 0:nb * D * nch].rearrange(
                            "p (i r e) -> p i r e", r=nb, e=D)
                    else:
                        dst = xpv[:, b0:b0 + nb, :]
                        src = xpb[:, 0:nb * D].rearrange("p (r e) -> p r e", e=D)
                    dma_c.dma_start(dst, src)
    nc.finalize()
    return nc


# ---------------- host entry ----------------
def _run(x, enc_mean, enc_var, dec_mean, eps, dt="bf16", mc_n=MC,
         trace=False, trace_kwargs=None):
    np_dt = {"f32": np.float32, "bf16": ml_dtypes.bfloat16}[dt]
    dt_dec = {"f32": F32, "bf16": BF16}[dt]
    p32, pD = make_packs(enc_mean, enc_var, dec_mean, np_dt)
    nc = build_kernel(p32, pD, dt_dec=dt_dec, mc_n=mc_n)

    x = np.asarray(x, np.float32)
    eps = np.asarray(eps, np.float32)
    w32_a = p32.finalize()
    wd_a = pD.finalize()
    in_maps = []
    for c in range(NCORES):
        sl = slice(c * BS, (c + 1) * BS)
        in_maps.append({
            "x_s": np.ascontiguousarray(x[sl]),
            "eps_s": np.ascontiguousarray(eps[:mc_n, sl]),
            "w32": w32_a,
            "wd": wd_a,
        })
    res = run_bass_kernel_spmd(nc, in_maps, list(range(NCORES)),
                               trace=trace, **(trace_kwargs or {}))
    em = np.concatenate([r["em_o"] for r in res.results], axis=0)
    ev = np.concatenate([r["ev_o"] for r in res.results], axis=0)
    z = np.concatenate([r["z_o"] for r in res.results], axis=1)
    xp = np.concatenate([r["xp_o"] for r in res.results], axis=1)
    return (em, ev, z, xp), res


def kernel(x, enc_mean, enc_var, dec_mean, eps):
    outs, _ = _run(x, enc_mean, enc_var, dec_mean, eps,
                   dt=os.environ.get("KERNEL_DT", "bf16"))
    return outs
